# revision 18
# baseline (speedup 1.0000x reference)
"""Trainium2 Bass kernel for nn_DistributionLossWithLabel.

Reference computation (B=8192, C=64):
    lq = log(q); lp = log(p)
    positive[i] = mean_c p[i,c]*(lp[i,c]-lq[i,c])
    kl[i,j]     = (1/C) * sum_c p[j,c]*(lp[j,c] - lq[i,c])
    negative[i] = sum_j kl[i,j]*(2 - L[i,j])
    loss        = sum_i positive[i]/negative[i]

Device reformulation (rows i sharded 8 ways):
    2 - L = 1 + L' with L' = 1 - L in {0,1}.  The "1" part of negative is
    label-free and computed exactly on host.  The label part needs
        S[m,i] = sum_j paug[j,m] * L'[i,j]
    with paug = [64*p | 32*wsum_hi | 32*wsum_lo] (66 fp8e4m3 columns,
    wsum[j] = sum_c p[j,c]*log p[j,c] carried hi/lo to kill fp8 rounding
    of the dominant term; scales undone on host).  The device computes
    ONLY S: a [66 x 8192] @ [8192 x 1024] fp8 matmul per core, streaming
    L'^T straight from HBM through the PE in DoubleRow mode (2 fp8
    MACs/cell/cycle, contraction 256 rows per pass).  The 8192x8192
    labels matrix is read exactly once as fp8 (8MB/core) and every
    O(B*C) op (logs, positive, final reduction) happens on host in f64.
"""

import sys

if "/opt/trn_rl_repo" not in sys.path:
    sys.path.insert(0, "/opt/trn_rl_repo")

import ml_dtypes
import numpy as np

import concourse.bass as bass
import concourse.tile as tile
from concourse import bacc, mybir

FP = mybir.dt.float32
F8 = mybir.dt.float8e4
F8NP = ml_dtypes.float8_e4m3

B_FULL = 8192
C = 64
N_CORES = 8
SHARD = B_FULL // N_CORES          # 1024 rows i per core
NSUB = B_FULL // 128               # 64 j-subchunks of 128
NAUG = 66                          # 64 p cols + wsum hi + lo
WPAD = 80                          # padded so the DoubleRow LDW step is 16-aligned
SP = 64.0                          # fp8 scale for the p columns
SW = 32.0                          # fp8 scale for the wsum hi/lo columns

# j-subchunks per DMA tile: small first tiles so the first matmuls start
# early, tapered last tiles so the PE (which outruns the stream) never
# waits on a big late tile + its completion latency.
TILE_SUBS = (2, 2, 4, 8, 8, 8, 8, 8, 4, 4, 4, 2, 2)
assert sum(TILE_SUBS) == NSUB


def build_nc(debug=False):
    """Single-core SPMD program: mps[m,i] = sum_j paug[j,m] * L'T[j,i]."""
    nc = bacc.Bacc("TRN2", target_bir_lowering=False, debug=debug)

    # paug packed [128, 64, 80]: [p, s, m] = paug[j=s*128+p, m] (fp8, cols
    # 66..79 zero-padded)
    paug_d = nc.dram_tensor("paug", [128, NSUB * WPAD], F8, kind="ExternalInput")
    # L'^T packed [128, 64, 1024]: [p, s, i] = L'[core*1024+i, j=s*128+p]
    lab_d = nc.dram_tensor("lab", [128, NSUB * SHARD], F8, kind="ExternalInput")
    out_d = nc.dram_tensor("out", [NAUG, SHARD], FP, kind="ExternalOutput")

    DR = mybir.MatmulPerfMode.DoubleRow

    with tile.TileContext(nc) as tc:
        nl8 = sum(1 for s in TILE_SUBS if s == 8)
        nl4 = sum(1 for s in TILE_SUBS if s == 4)
        nl2 = sum(1 for s in TILE_SUBS if s == 2)
        with (
            tc.tile_pool(name="wpool", bufs=1) as wp,
            # one slot per tile (whole L' stream fits in SBUF) so no DMA
            # ever waits on the PE releasing a slot
            tc.tile_pool(name="lpool", bufs=max(nl8, nl4, nl2)) as lp,
            tc.tile_pool(name="opool", bufs=1) as op,
            tc.tile_pool(name="ps", bufs=1, space="PSUM") as pp,
        ):
            paug_t = wp.tile([128, NSUB, WPAD], F8)
            pa = paug_d.ap()

            def paug_dma(eng, s0, s1):
                eng.dma_start(
                    out=paug_t[:, s0:s1, :],
                    in_=pa[:, s0 * WPAD : s1 * WPAD].rearrange(
                        "p (s m) -> p s m", m=WPAD
                    ),
                )

            # head (subchunks 0-3) on the fast HWDGE ring as the very
            # first DMA — it gates the first LDWEIGHTS; the rest rides
            # the SWDGE ring off the critical path so both HWDGE rings
            # carry the big L' stream uninterrupted.
            paug_dma(nc.sync, 0, 4)
            paug_dma(nc.gpsimd, 4, NSUB)

            mps = pp.tile([WPAD, SHARD], FP)
            la = lab_d.ap()
            off = 0
            for t, S in enumerate(TILE_SUBS):
                Lt = lp.tile([128, S, SHARD], F8, tag=f"L{S}")
                eng = nc.scalar if t % 2 == 0 else nc.sync
                eng.dma_start(
                    out=Lt[:],
                    in_=la[:, off * SHARD : (off + S) * SHARD].rearrange(
                        "p (s i) -> p s i", i=SHARD
                    ),
                )
                for ds in range(S // 2):
                    c0 = off + 2 * ds          # global first subchunk of pair
                    lw = paug_t[:, c0 : c0 + 2, :]
                    # last double-chunk: finish bank0 LAST so both PSUM
                    # drains overlap the final matmuls
                    hs = (1, 0) if c0 == NSUB - 2 else (0, 1)
                    for h in hs:
                        nc.tensor.matmul(
                            mps[:, h * 512 : (h + 1) * 512],
                            lw,
                            Lt[:, 2 * ds : 2 * ds + 2, h * 512 : (h + 1) * 512],
                            start=(c0 == 0),
                            stop=(c0 == NSUB - 2),
                            perf_mode=DR,
                        )
                off += S

            # drain PSUM with two engines in parallel, out-DMA on two
            # queues; bank1 (cols 512+) finished first thanks to the
            # h-swap above, so its copy overlaps the final matmul.
            osb = op.tile([NAUG, SHARD], FP)
            nc.scalar.copy(osb[:, 512:1024], mps[0:NAUG, 512:1024])
            nc.vector.tensor_copy(osb[:, 0:512], mps[0:NAUG, 0:512])
            nc.scalar.dma_start(out=out_d.ap()[:, 512:1024], in_=osb[:, 512:1024])
            nc.sync.dma_start(out=out_d.ap()[:, 0:512], in_=osb[:, 0:512])

    nc.compile()
    return nc


_NC_CACHE = {}


def _get_nc():
    if "nc" not in _NC_CACHE:
        _NC_CACHE["nc"] = build_nc()
    return _NC_CACHE["nc"]


def prepare(q, p, labels_matrix):
    """Host prep: fp8 operands in the packed on-chip layout + f64 epilogue
    context. All O(B*C); the only O(B^2) work is the fp8 cast/transpose."""
    q = np.asarray(q, dtype=np.float32)
    p = np.asarray(p, dtype=np.float32)
    L = np.asarray(labels_matrix, dtype=np.float32)

    p64 = p.astype(np.float64)
    lp = np.log(p64)
    lq = np.log(q.astype(np.float64))
    w = p64 * lp

    wsum = w.sum(axis=1) * SW                      # [B] ~ -130
    hi = wsum.astype(np.float32).astype(F8NP)
    lo = (wsum - hi.astype(np.float64)).astype(np.float32).astype(F8NP)
    paug8 = np.zeros((B_FULL, WPAD), dtype=F8NP)
    paug8[:, 0:64] = (p64 * SP).astype(np.float32).astype(F8NP)
    paug8[:, 64] = hi
    paug8[:, 65] = lo
    paug_packed = np.ascontiguousarray(
        paug8.reshape(NSUB, 128, WPAD).transpose(1, 0, 2)
    ).reshape(128, NSUB * WPAD)

    L8 = (1.0 - L).astype(F8NP)                    # [i, j], {0,1} exact
    in_maps = []
    for k in range(N_CORES):
        Lt = L8[k * SHARD : (k + 1) * SHARD, :].T  # [8192 j, 1024 i]
        lab_k = np.ascontiguousarray(
            Lt.reshape(NSUB, 128, SHARD).transpose(1, 0, 2)
        ).reshape(128, NSUB * SHARD)
        in_maps.append({"paug": paug_packed, "lab": lab_k})

    positive = (p64 * (lp - lq)).mean(axis=1)                  # [B]
    base = (w.sum() - lq @ p64.sum(axis=0)) / C                # [B]
    ctx = {"positive": positive, "base": base, "lq": lq}
    return in_maps, ctx


def finalize(results, ctx):
    """negative[i] = base[i] + ((hi+lo)[i]/SW - sum_c lq[i,c]Dp[c,i]/SP)/C"""
    total = 0.0
    for k, r in enumerate(results):
        mps = r["out"].astype(np.float64)          # [66, 1024]
        dw = (mps[64, :] + mps[65, :]) / SW
        lqk = ctx["lq"][k * SHARD : (k + 1) * SHARD, :]
        dp = np.einsum("ic,ci->i", lqk, mps[0:64, :]) / SP
        neg = ctx["base"][k * SHARD : (k + 1) * SHARD] + (dw - dp) / C
        total += np.sum(ctx["positive"][k * SHARD : (k + 1) * SHARD] / neg)
    return np.float32(total)


def kernel(q, p, labels_matrix):
    from concourse.bass_utils import run_bass_kernel_spmd

    nc = _get_nc()
    in_maps, ctx = prepare(q, p, labels_matrix)
    res = run_bass_kernel_spmd(nc, in_maps, core_ids=list(range(N_CORES)))
    return finalize(res.results, ctx)
